# revision 1
# baseline (speedup 1.0000x reference)
"""CTC loss (keras ctc_batch_cost semantics) on Trainium2, 8-core data parallel.

Algorithm (per core, 64 examples):
  Linear-domain CTC forward with a constant per-step rescale K folded into the
  probabilities (p' = K*p, loss = T*log K - log(alpha_end)), parity-split
  lattice columns, and a wavefront over columns where each column's serial
  T-recurrence is ONE DVE tensor_tensor_scan (state = p*state + v, fp32 state).

  Data movement: y_pred [64,512,128] f32 is transposed per example on the PE
  (128x128 tiles) with the K-scale + bf16 downcast folded into the PSUM->SBUF
  copy on the scalar engine, stored to DRAM as yT [64*128, 512] bf16, and the
  48 label columns are fetched with indirect-DMA row gathers (1KB rows, one
  [64,1]-offset gather per label column; multi-offset gathers are broken on HW).

Shapes are hardcoded for B=512, T=512, C=128, L=48 (S=97), 8 cores.
"""

import sys

if "/opt/trn_rl_repo" not in sys.path:
    sys.path.insert(0, "/opt/trn_rl_repo")

import math

import numpy as np

import concourse.bacc as bacc
import concourse.bass as bass
import concourse.tile as tile
from concourse import mybir
from concourse.bass_utils import run_bass_kernel_spmd
from concourse.masks import make_identity

NCORES = 8
B, T, C, L = 512, 512, 128, 48
BL = B // NCORES  # 64 examples per core
BLANK = C - 1
K = 75.0  # per-step rescale; log K ~= 4.317, actual growth ~= -4.367/step
F32 = mybir.dt.float32
BF16 = mybir.dt.bfloat16
I32 = mybir.dt.int32
ALU = mybir.AluOpType
ACTF = mybir.ActivationFunctionType


def build_ctc_program(nc: bass.Bass, phases=3):
    y_pred = nc.dram_tensor("y_pred", [BL, T, C], F32, kind="ExternalInput").ap()
    y_true = nc.dram_tensor("y_true", [BL, L], I32, kind="ExternalInput").ap()
    out = nc.dram_tensor("out", [BL, 1], F32, kind="ExternalOutput").ap()

    with tile.TileContext(nc) as tc:
        _ctc_body(nc, tc, y_pred, y_true, out, phases)
    return out


def _ctc_body(nc, tc, y_pred, y_true, out, phases=3):
    TC = T // 128  # 4 t-chunks per example

    with (
        tc.tile_pool(name="const", bufs=1) as cpool,
        tc.tile_pool(name="ld", bufs=3) as ldpool,
        tc.tile_pool(name="ps", bufs=4, space="PSUM") as pspool,
        tc.tile_pool(name="yt16", bufs=3) as ytpool,
        tc.tile_pool(name="dram", bufs=1, space="DRAM") as dpool,
        tc.tile_pool(name="pg", bufs=48) as pgpool,
        tc.tile_pool(name="cols", bufs=4) as colpool,
        tc.tile_pool(name="work", bufs=4) as wpool,
        tc.tile_pool(name="fin", bufs=1) as fpool,
    ):
        # ---- constants / label-derived data ------------------------------
        ident = cpool.tile([128, 128], F32)
        make_identity(nc, ident[:])

        lab = cpool.tile([BL, L], I32)
        nc.sync.dma_start(out=lab[:], in_=y_true[:, :])

        bidx = cpool.tile([BL, L], I32)
        nc.gpsimd.iota(bidx[:], [[0, L]], base=0, channel_multiplier=C)

        # gather row index: b*C + label
        idx = cpool.tile([BL, L], I32)
        nc.vector.tensor_tensor(out=idx[:], in0=lab[:], in1=bidx[:], op=ALU.add)

        # skip mask m[b,i] = (lab[i] != lab[i-1]), m[:,0] = 0
        m = cpool.tile([BL, L], F32)
        nc.vector.memset(m[:, 0:1], 0.0)
        nc.vector.tensor_tensor(
            out=m[:, 1:L], in0=lab[:, 1:L], in1=lab[:, 0 : L - 1], op=ALU.not_equal
        )

        # ---- transpose + scale + downcast: yT[b*C+c, t] = K*y_pred[b,t,c] --
        yT = dpool.tile([BL * C, T], BF16)
        y4 = y_pred.rearrange("(g w) (a p) c -> g p w a c", w=2, p=128)
        yT4 = yT[:].rearrange("(g w c) t -> g c w t", w=2, c=C)
        for g in range(BL // 2):
            ysb = ldpool.tile([128, 2, TC, C], F32, tag="ysb")
            nc.sync.dma_start(out=ysb[:], in_=y4[g])
            yt16 = ytpool.tile([128, 2 * T], BF16, tag="yt16")
            for w in range(2):
                ps = pspool.tile([128, T], F32, tag="ps")  # exactly one PSUM bank
                for ch in range(TC):
                    nc.tensor.transpose(
                        ps[:, ch * 128 : (ch + 1) * 128], ysb[:, w, ch, :], ident[:]
                    )
                nc.scalar.activation(
                    out=yt16[:, w * T : (w + 1) * T], in_=ps[:], func=ACTF.Copy,
                    scale=K,
                )
            nc.sync.dma_start(out=yT4[g], in_=yt16[:].rearrange("c (w t) -> c w t", w=2))

        if phases < 2:
            dummy = fpool.tile([BL, 1], F32, tag="dummy")
            nc.vector.memset(dummy[:], 0.0)
            nc.sync.dma_start(out=out[:, :], in_=dummy[:])
            return

        # ---- gather lattice probability columns --------------------------
        # blank column (shared by all 49 blank lattice states)
        pb = cpool.tile([BL, T], BF16)
        yT3 = yT[:].rearrange("(b c) t -> b c t", c=C)
        nc.sync.dma_start(out=pb[:], in_=yT3[:, BLANK, :])

        pg = []  # label columns, one tile each so deps stay per-column
        for i in range(L):
            pgi = pgpool.tile([BL, T], BF16, tag="pg")
            nc.gpsimd.indirect_dma_start(
                out=pgi[:],
                out_offset=None,
                in_=yT[:],
                in_offset=bass.IndirectOffsetOnAxis(ap=idx[:, i : i + 1], axis=0),
            )
            pg.append(pgi)

        if phases < 3:
            dummy = fpool.tile([BL, 1], F32, tag="dummy")
            nc.vector.tensor_tensor(
                out=dummy[:], in0=pg[L - 1][:, 0:1], in1=pb[:, 0:1], op=ALU.add
            )
            nc.sync.dma_start(out=out[:, :], in_=dummy[:])
            return

        # ---- wavefront over lattice columns ------------------------------
        # column tiles [BL, T+1]: slot 0 = t=-1 boundary, slots 1..T = scan out
        lprev = colpool.tile([BL, T + 1], BF16, tag="lcol")
        nc.gpsimd.memset(lprev[:], 0.0)  # l_{-1} == 0

        acol = None
        for i in range(L + 1):
            # blank column a_i: a[t] = pb[t]*(a[t-1] + lprev[t-1])
            acol = colpool.tile([BL, T + 1], BF16, tag="acol")
            nc.scalar.activation(
                out=acol[:, 0:1], in_=m[:, 0:1], func=ACTF.Copy,
                scale=0.0, bias=1.0 if i == 0 else 0.0,
            )
            va = wpool.tile([BL, T], BF16, tag="va")
            nc.vector.tensor_tensor(
                out=va[:], in0=lprev[:, 0:T], in1=pb[:], op=ALU.mult
            )
            nc.vector.tensor_tensor_scan(
                out=acol[:, 1 : T + 1], data0=pb[:], data1=va[:],
                initial=1.0 if i == 0 else 0.0, op0=ALU.mult, op1=ALU.add,
            )
            if i == L:
                break

            # label column l_i: l[t] = pl[t]*(l[t-1] + a_i[t-1] + m_i*lprev[t-1])
            # m_i*lprev runs on the scalar engine, off the DVE critical chain
            gmask = wpool.tile([BL, T], BF16, tag="gmask")
            nc.scalar.activation(
                out=gmask[:], in_=lprev[:, 0:T], func=ACTF.Copy,
                scale=m[:, i : i + 1],
            )

            lcol = colpool.tile([BL, T + 1], BF16, tag="lcol")
            nc.scalar.activation(
                out=lcol[:, 0:1], in_=m[:, 0:1], func=ACTF.Copy, scale=0.0, bias=0.0,
            )
            x = wpool.tile([BL, T], BF16, tag="x")
            nc.vector.tensor_tensor(
                out=x[:], in0=gmask[:], in1=acol[:, 0:T], op=ALU.add
            )
            vl = wpool.tile([BL, T], BF16, tag="vl")
            nc.vector.tensor_tensor(
                out=vl[:], in0=x[:], in1=pg[i][:], op=ALU.mult
            )
            nc.vector.tensor_tensor_scan(
                out=lcol[:, 1 : T + 1], data0=pg[i][:], data1=vl[:],
                initial=0.0, op0=ALU.mult, op1=ALU.add,
            )
            lprev = lcol

        # ---- finalize: loss = T*log K - log(a_L[T] + l_{L-1}[T]) ---------
        z = fpool.tile([BL, 1], F32)
        nc.vector.tensor_tensor(
            out=z[:], in0=acol[:, T : T + 1], in1=lprev[:, T : T + 1], op=ALU.add
        )
        logz = fpool.tile([BL, 1], F32)
        nc.scalar.activation(out=logz[:], in_=z[:], func=ACTF.Ln)
        loss = fpool.tile([BL, 1], F32)
        nc.scalar.activation(
            out=loss[:], in_=logz[:], func=ACTF.Copy,
            scale=-1.0, bias=float(T * math.log(K)),
        )
        nc.sync.dma_start(out=out[:, :], in_=loss[:])


_CACHE: dict = {}


def _get_program():
    if "nc" not in _CACHE:
        nc = bacc.Bacc("TRN2", target_bir_lowering=False, debug=False)
        build_ctc_program(nc)
        nc.compile()
        _CACHE["nc"] = nc
    return _CACHE["nc"]


def kernel(y_true: np.ndarray, y_pred: np.ndarray) -> np.ndarray:
    nc = _get_program()
    yt = np.ascontiguousarray(np.asarray(y_true).astype(np.int32))
    yp = np.ascontiguousarray(np.asarray(y_pred, dtype=np.float32))
    in_maps = [
        {
            "y_true": yt[c * BL : (c + 1) * BL],
            "y_pred": yp[c * BL : (c + 1) * BL],
        }
        for c in range(NCORES)
    ]
    res = run_bass_kernel_spmd(nc, in_maps, list(range(NCORES)))
    return np.concatenate([res.results[c]["out"] for c in range(NCORES)], axis=0)



# revision 2
# speedup vs baseline: 5.3049x; 5.3049x over previous
"""CTC loss (keras ctc_batch_cost semantics) on Trainium2, 8-core data parallel.

Algorithm (per core, 64 examples):
  Linear-domain CTC forward with per-step rescale K folded into the
  probabilities (p' = K*p, loss = T*log K - log(sum)), computed
  BIDIRECTIONALLY over the 97-column lattice: a forward alpha chain over
  columns s=0..48 runs on partitions 0-63 while an independent backward
  beta chain over columns s=96..49 runs on partitions 64-127, packed into
  the same [128, T] DVE instructions. The chains meet at the s=48|49 cut:
    loss_path_sum = sum_t (alpha[t-1,47]*sk49 + alpha[t-1,48]) * beta'[t,49].
  This halves the serial scan chain (49 scans of T=512 instead of 97).

  Each column is ONE tensor_tensor_scan: state = (data0 + state) * data1
  (op0=add, op1=mult), so the old per-column multiply TTs are folded away.
  The label-column input x = m*lprev + acol is ONE scalar_tensor_tensor
  with the skip mask as a per-partition scalar.

  Data movement: y_pred transposed per 4-example group on the PE
  (128x128 tiles), K-scale + bf16 downcast in the PSUM->SBUF copy on the
  scalar engine, stored to DRAM yT [64*128, 512] bf16. Lattice columns
  fetched with 24 PACKED indirect-DMA row gathers: fwd label k rows on
  partitions 0-63, bwd label 47-k rows on partitions 64-127. Backward
  columns are time-reversed by scalar-engine copies with negative-stride
  access patterns.

Shapes hardcoded for B=512, T=512, C=128, L=48 (S=97), 8 cores.
"""

import sys

if "/opt/trn_rl_repo" not in sys.path:
    sys.path.insert(0, "/opt/trn_rl_repo")

import math

import numpy as np

import concourse.bacc as bacc
import concourse.bass as bass
import concourse.tile as tile
from concourse import mybir
from concourse.bass_utils import run_bass_kernel_spmd
from concourse.masks import make_identity

NCORES = 8
B, T, C, L = 512, 512, 128, 48
BL = B // NCORES  # 64 examples per core
BLANK = C - 1
K = 75.0
NP = L // 2  # 24 label steps per direction
F32 = mybir.dt.float32
BF16 = mybir.dt.bfloat16
I32 = mybir.dt.int32
ALU = mybir.AluOpType
ACTF = mybir.ActivationFunctionType


def _rev(ap):
    """Free-dim-reversed view of a [P, N] AP (N contiguous)."""
    dims = [list(d) for d in ap.ap]
    assert len(dims) == 2 and dims[1][0] == 1
    n = dims[1][1]
    return bass.AP(ap.tensor, ap.offset + (n - 1), [dims[0], [-1, n]])


def build_ctc_program(nc: bass.Bass):
    y_pred = nc.dram_tensor("y_pred", [BL, T, C], F32, kind="ExternalInput").ap()
    y_true = nc.dram_tensor("y_true", [BL, L], I32, kind="ExternalInput").ap()
    out = nc.dram_tensor("out", [BL, 1], F32, kind="ExternalOutput").ap()

    with tile.TileContext(nc) as tc:
        _ctc_body(nc, tc, y_pred, y_true, out)
    return out


def _ctc_body(nc, tc, y_pred, y_true, out):
    yT = nc.dram_tensor("yT", [BL * C, T], BF16, kind="Internal").ap()

    with (
        tc.tile_pool(name="const", bufs=1) as cpool,
        tc.tile_pool(name="ld", bufs=3) as ldpool,
        tc.tile_pool(name="ps", bufs=8, space="PSUM") as pspool,
        tc.tile_pool(name="yt16", bufs=3) as ytpool,
        tc.tile_pool(name="pl", bufs=26) as plpool,
        tc.tile_pool(name="d", bufs=26) as dpool,
        tc.tile_pool(name="col", bufs=50) as colpool,
        tc.tile_pool(name="work", bufs=3) as wpool,
        tc.tile_pool(name="fin", bufs=1) as fpool,
    ):
        # ---- constants / label-derived data ------------------------------
        ident = cpool.tile([128, 128], F32)
        make_identity(nc, ident[:])

        lnwarm = cpool.tile([1, 1], F32)
        nc.vector.memset(lnwarm[:], 1.0)
        nc.scalar.activation(out=lnwarm[:], in_=lnwarm[:], func=ACTF.Ln)

        lab = cpool.tile([BL, L], I32)
        nc.scalar.dma_start(out=lab[:], in_=y_true[:, :])

        bidx = cpool.tile([BL, L], I32)
        nc.gpsimd.iota(bidx[:], [[0, L]], base=0, channel_multiplier=C)

        idx = cpool.tile([BL, L], I32)
        nc.vector.tensor_tensor(out=idx[:], in0=lab[:], in1=bidx[:], op=ALU.add)

        # skip mask m[b,i] = (lab[i] != lab[i-1]), m[:,0] = 0  (f32, [64, L])
        m = cpool.tile([BL, L], F32)
        nc.vector.memset(m[:, 0:1], 0.0)
        nc.vector.tensor_tensor(
            out=m[:, 1:L], in0=lab[:, 1:L], in1=lab[:, 0 : L - 1], op=ALU.not_equal
        )

        # packed per-step mask M [128, NP]: fwd m[:,k] on 0-63,
        # bwd skbw[:,k] = m[:,48-k] (k>=1, col 0 = 0) on 64-127
        M = cpool.tile([128, NP], F32)
        nc.scalar.activation(out=M[0:BL, :], in_=m[:, 0:NP], func=ACTF.Copy)
        skb = cpool.tile([BL, NP], F32)
        nc.vector.memset(skb[:, 0:1], 0.0)
        # skbw[:,k] = (lab[:,48-k] != lab[:,47-k]) for k=1..23
        lab_a = lab[:, 25:L]  # cols 25..47 -> reversed = 47..25 = lab[:,48-k]
        lab_b = lab[:, 24 : L - 1]  # cols 24..46 -> reversed = 46..24 = lab[:,47-k]
        nc.vector.tensor_tensor(
            out=skb[:, 1:NP], in0=_rev(lab_a), in1=_rev(lab_b), op=ALU.not_equal
        )
        nc.scalar.dma_start(out=M[BL:128, :], in_=skb[:])

        # packed gather rows idx2 [128, NP]: fwd idx[:,k] on 0-63,
        # bwd idx[:,47-k] on 64-127
        idx2 = cpool.tile([128, NP], I32)
        nc.vector.tensor_tensor(
            out=idx2[0:BL, :], in0=lab[:, 0:NP], in1=bidx[:, 0:NP], op=ALU.add
        )
        idxb = cpool.tile([BL, NP], I32)
        nc.vector.tensor_tensor(
            out=idxb[:], in0=_rev(lab[:, NP:L]), in1=bidx[:, 0:NP], op=ALU.add
        )
        nc.scalar.dma_start(out=idx2[BL:128, :], in_=idxb[:])

        # ---- lattice column tiles (slot 0 = t-1 boundary) ----------------
        lzero = colpool.tile([128, T + 1], BF16, tag="lzero")
        nc.gpsimd.memset(lzero[:], 0.0)
        acols = []
        for k in range(NP + 1):
            a = colpool.tile([128, T + 1], BF16, tag="col")
            nc.gpsimd.memset(a[:, 0:1], 1.0 if k == 0 else 0.0)
            acols.append(a)
        lcols = []
        for k in range(NP):
            lc = colpool.tile([128, T + 1], BF16, tag="col")
            nc.gpsimd.memset(lc[:, 0:1], 0.0)
            lcols.append(lc)

        # ---- transpose + scale + downcast: yT[e*C+c, t] = K*y_pred[e,t,c] -
        TC = T // 128
        G = BL // 4  # 16 groups of 4 examples
        y4 = y_pred.rearrange("(g w) (a p) c -> g p w a c", w=4, p=128)
        yT4 = yT.rearrange("(g w c) t -> g c w t", w=4, c=C)
        for g in range(G):
            ysb = ldpool.tile([128, 4, TC, C], F32, tag="ysb")
            nc.sync.dma_start(out=ysb[:], in_=y4[g])
            yt16 = ytpool.tile([128, 4 * T], BF16, tag="yt16")
            for w in range(4):
                ps = pspool.tile([128, T], F32, tag="ps")
                for ch in range(TC):
                    nc.tensor.transpose(
                        ps[:, ch * 128 : (ch + 1) * 128], ysb[:, w, ch, :], ident[:]
                    )
                nc.scalar.activation(
                    out=yt16[:, w * T : (w + 1) * T], in_=ps[:], func=ACTF.Copy,
                    scale=K,
                )
            nc.sync.dma_start(
                out=yT4[g], in_=yt16[:].rearrange("c (w t) -> c w t", w=4)
            )

        # ---- blank column (shared): fwd natural on 0-63, bwd reversed ----
        yTr = yT.rearrange("(e c) t -> e c t", c=C)
        pbraw = cpool.tile([128, T], BF16)
        nc.scalar.dma_start(out=pbraw[0:BL, :], in_=yTr[:, BLANK, :])
        nc.scalar.dma_start(out=pbraw[BL:128, :], in_=yTr[:, BLANK, :])
        pbk = cpool.tile([128, T], BF16)
        nc.scalar.activation(out=pbk[0:BL, :], in_=pbraw[0:BL, :], func=ACTF.Copy)
        nc.scalar.activation(
            out=pbk[BL:128, :], in_=_rev(pbraw[BL:128, :]), func=ACTF.Copy
        )

        # ---- packed label-column gathers + bwd time reversal -------------
        dcols = []
        for k in range(NP):
            pl = plpool.tile([128, T], BF16, tag="pl")
            nc.gpsimd.indirect_dma_start(
                out=pl[:],
                out_offset=None,
                in_=yT[:],
                in_offset=bass.IndirectOffsetOnAxis(ap=idx2[:, k : k + 1], axis=0),
            )
            d = dpool.tile([128, T], BF16, tag="d")
            nc.scalar.activation(out=d[0:BL, :], in_=pl[0:BL, :], func=ACTF.Copy)
            nc.scalar.activation(
                out=d[BL:128, :], in_=_rev(pl[BL:128, :]), func=ACTF.Copy
            )
            dcols.append(d)

        # ---- bidirectional wavefront -------------------------------------
        lprev = lzero
        acol = None
        for k in range(NP + 1):
            acol = acols[k]
            nc.vector.tensor_tensor_scan(
                out=acol[:, 1 : T + 1], data0=lprev[:, 0:T], data1=pbk[:],
                initial=1.0 if k == 0 else 0.0, op0=ALU.add, op1=ALU.mult,
            )
            if k == NP:
                break
            x = wpool.tile([128, T], BF16, tag="x")
            nc.vector.scalar_tensor_tensor(
                out=x[:], in0=lprev[:, 0:T], scalar=M[:, k : k + 1],
                in1=acol[:, 0:T], op0=ALU.mult, op1=ALU.add,
            )
            lcol = lcols[k]
            nc.vector.tensor_tensor_scan(
                out=lcol[:, 1 : T + 1], data0=x[:], data1=dcols[k][:],
                initial=0.0, op0=ALU.add, op1=ALU.mult,
            )
            lprev = lcol

        # ---- finalize at the s=48|49 cut ---------------------------------
        # beta' col 49 lives on partitions 64-127 of lprev (= m_23); move down.
        bbt = fpool.tile([BL, T + 1], BF16)
        nc.scalar.dma_start(out=bbt[:], in_=lprev[BL:128, :])
        # xf[j] = m24 * l23[t-1] + a24[t-1],  t = j+1, slots 1..511
        xf = fpool.tile([BL, T - 1], BF16)
        nc.vector.scalar_tensor_tensor(
            out=xf[:], in0=lprev[0:BL, 1:T], scalar=m[:, NP : NP + 1],
            in1=acol[0:BL, 1:T], op0=ALU.mult, op1=ALU.add,
        )
        # z = sum_t xf * beta'[t,49];  beta'[t,49] = bbt[:, 512-t] -> slots 511..1
        bb_ap = bass.AP(
            bbt[:].tensor, bbt[:].offset + (T - 1),
            [[list(d) for d in bbt[:].ap][0], [-1, T - 1]],
        )
        prodj = fpool.tile([BL, T - 1], BF16)
        z = fpool.tile([BL, 1], F32)
        nc.vector.scalar_tensor_tensor(
            out=prodj[:], in0=xf[:], scalar=1.0, in1=bb_ap,
            op0=ALU.mult, op1=ALU.mult, accum_out=z[:],
        )
        logz = fpool.tile([BL, 1], F32)
        nc.scalar.activation(out=logz[:], in_=z[:], func=ACTF.Ln)
        loss = fpool.tile([BL, 1], F32)
        nc.scalar.activation(
            out=loss[:], in_=logz[:], func=ACTF.Copy,
            scale=-1.0, bias=float(T * math.log(K)),
        )
        nc.sync.dma_start(out=out[:, :], in_=loss[:])


_CACHE: dict = {}


def _get_program():
    if "nc" not in _CACHE:
        nc = bacc.Bacc("TRN2", target_bir_lowering=False, debug=False)
        build_ctc_program(nc)
        nc.compile()
        _CACHE["nc"] = nc
    return _CACHE["nc"]


def kernel(y_true: np.ndarray, y_pred: np.ndarray) -> np.ndarray:
    nc = _get_program()
    yt = np.ascontiguousarray(np.asarray(y_true).astype(np.int32))
    yp = np.ascontiguousarray(np.asarray(y_pred, dtype=np.float32))
    in_maps = [
        {
            "y_true": yt[c * BL : (c + 1) * BL],
            "y_pred": yp[c * BL : (c + 1) * BL],
        }
        for c in range(NCORES)
    ]
    res = run_bass_kernel_spmd(nc, in_maps, list(range(NCORES)))
    return np.concatenate([res.results[c]["out"] for c in range(NCORES)], axis=0)
